# revision 6
# baseline (speedup 1.0000x reference)
"""Trainium2 Bass kernel for CausalGraphNetwork.

Computes, for x = step_sequence [B=2, N=512, H=256]:
    h  = relu(x @ W_gc1.T + b_gc1)
    f  = relu(h @ W_gc2.T + b_gc2)
    a  = f @ Wa.T            (Wa = W_ep1[:, :H])
    c  = f @ Wb.T            (Wb = W_ep1[:, H:])
    e[b,i,j,:] = relu(a[b,i,:] + c[b,j,:] + b_ep1)
    scores = sigmoid(e @ w_ep2 + b_ep2) * strict_lower_mask

Strategy (8 NeuronCores, SPMD single program):
  - Core d owns batch d//4, rows i = 4k + (d%4), k = 0..127.  The
    interleaved row assignment makes the causal work profile identical
    on every core, so one program serves all cores.
  - Everything on-chip in transposed layout (h on partitions):
    hT/fT/cT computed with fp32 matmuls; per row, e-chunks are built
    with a per-partition-bias tensor_scalar (relu fused via op1=max)
    and reduced over h by M=1 matmuls (lhsT = w_ep2 chunk) into PSUM.
  - PSUM writes for M=1 must land on 32-aligned partitions, so 4 rows
    share a bank at partitions {0,32,64,96}; sigmoid (ACT) drains a
    whole bank to SBUF, and a strided-partition DMA extracts the rows.
  - Causality: row k only computes j < Jext(k) = 64*ceil((k+1)/16);
    the host applies the exact strict-lower mask after gathering.
"""

import math

import numpy as np

import bass_rust
import concourse.bass as bass
import concourse.mybir as mybir
import concourse.tile as tile
from concourse.bass_utils import run_bass_kernel_spmd
from concourse.vector_clock import ScopedClock

B, N, H = 2, 512, 256
NCORES = 8
R = 128  # rows per core
F32 = mybir.dt.float32
BF16 = mybir.dt.bfloat16
AF = mybir.ActivationFunctionType
ALU = mybir.AluOpType

# e-tile dtype for the pairwise stage
E_DT = F32
# engine for e-generation per h-chunk: "vector", "gpsimd", or "scalar"
E_ENGINES = ("vector", "gpsimd")


def jext(k: int) -> int:
    """Causal column extent for local row k (covers i = 4k+3 worst case)."""
    return min(N, 64 * math.ceil((k + 1) / 16))


class _TC(tile.TileContext):
    """TileContext variant for a walrus build that only supports ONE sem
    wait per instruction: split multi-wait instructions by hoisting the
    extra waits onto NOPs inserted just before them."""

    MAXW = 1

    def _split_waits_in_list(self, insts):
        out = []
        for inst in insts:
            si = inst.sync_info
            waits = list(si.on_wait) if si is not None else []
            if len(waits) > self.MAXW:
                rest, keep = waits[: -self.MAXW], waits[-self.MAXW :]
                for i in range(0, len(rest), self.MAXW):
                    nop = mybir.InstNoOp(
                        name=self.nc.get_next_instruction_name(),
                        engine=inst.engine,
                        bass_nofuse=True,
                        sync_info=bass_rust.SyncInfo(
                            on_wait=rest[i : i + self.MAXW], on_update=[]
                        ),
                    )
                    out.append(nop)
                inst.sync_info = bass_rust.SyncInfo(
                    on_wait=keep, on_update=list(si.on_update)
                )
            out.append(inst)
        return out

    def _lower_ordered_insts(self, ordered):
        for bb_name in list(ordered.keys()):
            ordered[bb_name] = self._split_waits_in_list(ordered[bb_name])
        return super()._lower_ordered_insts(ordered)

    def _drain_and_barrier(self, tick_clock, wait_clock):
        drain_inst = self.nc.sync.drain()
        wait_clock.add_sem_waits(
            drain_inst.ins, ScopedClock({None: tick_clock.global_clock})
        )
        si = drain_inst.ins.sync_info
        waits = list(si.on_wait) if si is not None else []
        if len(waits) > self.MAXW:
            drain_inst.ins.sync_info = bass_rust.SyncInfo(
                on_wait=waits[: self.MAXW], on_update=list(si.on_update)
            )
            rest = waits[self.MAXW :]
            for i in range(0, len(rest), self.MAXW):
                nop = self.nc.sync.nop(nofuse=True, hint=f"dw{i}")
                nop.ins.sync_info = bass_rust.SyncInfo(
                    on_wait=rest[i : i + self.MAXW], on_update=[]
                )
        self.nc.all_engine_barrier()
        assert self.sems is not None
        popped = self.nc._tile_sem_poison_stack.pop()
        assert popped is self._sem_poison
        self.nc.clear_and_free_semaphores(list(self.sems.allocated().values()))
        self.nc.all_engine_barrier()


def _egen(nc, engine: str, out_ap, in_ap, bias_col):
    """e = relu(in + bias) with per-partition bias, on the chosen engine."""
    if engine == "vector":
        nc.vector.tensor_scalar(
            out=out_ap, in0=in_ap, scalar1=bias_col, scalar2=0.0,
            op0=ALU.add, op1=ALU.max,
        )
    elif engine == "gpsimd":
        nc.gpsimd.tensor_scalar(
            out=out_ap, in0=in_ap, scalar1=bias_col, scalar2=0.0,
            op0=ALU.add, op1=ALU.max,
        )
    elif engine == "scalar":
        nc.scalar.activation(out_ap, in_ap, AF.Relu, bias=bias_col)
    else:
        raise ValueError(engine)


def _body(nc, wpool, epool, scpool, banks,
          xt, y, wts, b1t, b2t, bep1t, wep2t, bep2t, NT):
    bank_rr = [0]

    def next_bank(tn):
        kk = bank_rr[0] % 8
        bank_rr[0] += 1
        return banks[kk][:, 0:tn]
    xts = []
    for c in range(2):
        t = wpool.tile([128, NT], F32, name=f"xt_{c}", tag=f"xt_{c}")
        nc.sync.dma_start(t[:, :], xt[c * 128 : (c + 1) * 128, :])
        xts.append(t)

    # ---- upstream: hT -> fT -> (cT, aTp), all [o-chunk, tokens] ----
    def layer(dst_tiles, src_tiles, wname, bias_tile, func, tslices):
        for oc in range(2):
            for (t0, tn) in tslices:
                ps = next_bank(tn)
                for kc in range(2):
                    nc.tensor.matmul(
                        ps,
                        lhsT=wts[(wname, kc)][:, oc * 128 : (oc + 1) * 128],
                        rhs=src_tiles[kc][:, t0 : t0 + tn],
                        start=(kc == 0), stop=(kc == 1),
                    )
                dst = dst_tiles[oc][:, t0 : t0 + tn]
                bias_col = bias_tile[:, oc : oc + 1] if bias_tile is not None else None
                if oc == 0:
                    if func == "relu":
                        nc.scalar.activation(dst, ps, AF.Relu, bias=bias_col)
                    elif func == "addbias":
                        nc.scalar.activation(dst, ps, AF.Identity, bias=bias_col)
                    else:
                        nc.scalar.copy(dst, ps)
                else:
                    if func == "relu":
                        nc.vector.tensor_scalar(
                            out=dst, in0=ps, scalar1=bias_col,
                            scalar2=0.0, op0=ALU.add, op1=ALU.max)
                    elif func == "addbias":
                        nc.vector.tensor_scalar(
                            out=dst, in0=ps, scalar1=bias_col,
                            scalar2=None, op0=ALU.add)
                    else:
                        nc.vector.tensor_copy(dst, ps)

    hts = [wpool.tile([128, NT], F32, name=f"ht_{c}", tag=f"ht_{c}") for c in range(2)]
    layer(hts, xts, "w1t", b1t, "relu", [(0, 512), (512, R)])
    fts = [wpool.tile([128, NT], F32, name=f"ft_{c}", tag=f"ft_{c}") for c in range(2)]
    layer(fts, hts, "w2t", b2t, "relu", [(0, 512), (512, R)])
    cts = [wpool.tile([128, N], E_DT, name=f"ct_{c}", tag=f"ct_{c}") for c in range(2)]
    layer(cts, fts, "wbt", None, "copy", [(0, 512)])

    # aTp = Wa-projection of own tokens + b_ep1
    ats = [wpool.tile([128, R], F32, name=f"at_{c}", tag=f"at_{c}") for c in range(2)]
    for oc in range(2):
        ps = next_bank(R)
        for kc in range(2):
            nc.tensor.matmul(
                ps,
                lhsT=wts[("wat", kc)][:, oc * 128 : (oc + 1) * 128],
                rhs=fts[kc][:, 512 : 512 + R],
                start=(kc == 0), stop=(kc == 1),
            )
        dst = ats[oc][:, :]
        bias_col = bep1t[:, oc : oc + 1]
        if oc == 0:
            nc.scalar.activation(dst, ps, AF.Identity, bias=bias_col)
        else:
            nc.vector.tensor_scalar(
                out=dst, in0=ps, scalar1=bias_col,
                scalar2=None, op0=ALU.add)

    # ---- pairwise stage ----
    for g in range(4):
        for kk in range(8):
            jb = jext(32 * g + 4 * kk)
            ps = banks[kk]
            for u in range(4):
                k = 32 * g + 4 * kk + u
                for c in range(2):
                    e = epool.tile([128, N], E_DT, name=f"e{c}", tag=f"e{c}", bufs=3)
                    _egen(nc, E_ENGINES[c],
                          e[:, 0:jb], cts[c][:, 0:jb], ats[c][:, k : k + 1])
                    nc.tensor.matmul(
                        ps[32 * u : 32 * u + 1, 0:jb],
                        lhsT=wep2t[:, c : c + 1],
                        rhs=e[:, 0:jb],
                        start=(c == 0), stop=(c == 1),
                        tile_position=(0, 32 * u),
                    )
            # epilogue: sigmoid whole bank -> sbuf, strided-partition DMA out
            sc = scpool.tile([128, N], F32, name="sc", tag="sc")
            nc.scalar.activation(
                sc[:, 0:jb], ps[:, 0:jb], AF.Sigmoid, bias=bep2t[:, 0:1]
            )
            r0 = 32 * g + 4 * kk
            nc.sync.dma_start(y[r0 : r0 + 4, 0:jb], sc[0:128:32, 0:jb])


def build_nc(reps: int = 1) -> bass.Bass:
    nc = bass.Bass("TRN2", target_bir_lowering=False, debug=False)

    NT = N + R  # 640 token columns: 512 shared j-tokens + 128 own i-tokens

    xt = nc.dram_tensor("xt", [H, NT], F32, kind="ExternalInput")
    w1t = nc.dram_tensor("w1t", [H, H], F32, kind="ExternalInput")
    w2t = nc.dram_tensor("w2t", [H, H], F32, kind="ExternalInput")
    wat = nc.dram_tensor("wat", [H, H], F32, kind="ExternalInput")
    wbt = nc.dram_tensor("wbt", [H, H], F32, kind="ExternalInput")
    b1 = nc.dram_tensor("b1", [128, 2], F32, kind="ExternalInput")
    b2 = nc.dram_tensor("b2", [128, 2], F32, kind="ExternalInput")
    bep1 = nc.dram_tensor("bep1", [128, 2], F32, kind="ExternalInput")
    wep2 = nc.dram_tensor("wep2", [128, 2], F32, kind="ExternalInput")
    bep2 = nc.dram_tensor("bep2", [128, 1], F32, kind="ExternalInput")
    y = nc.dram_tensor("y", [R, N], F32, kind="ExternalOutput")

    with _TC(nc) as tc:
        with tc.tile_pool(name="const", bufs=1) as cpool, \
             tc.tile_pool(name="work", bufs=2) as wpool, \
             tc.tile_pool(name="epool", bufs=3) as epool, \
             tc.tile_pool(name="scpool", bufs=4) as scpool:

            # ---- constants (loaded once, reused across reps) ----
            wts = {}
            for nm, dram in (("w1t", w1t), ("w2t", w2t), ("wat", wat), ("wbt", wbt)):
                for c in range(2):
                    t = cpool.tile([128, H], F32, name=f"{nm}_{c}")
                    nc.sync.dma_start(t[:, :], dram[c * 128 : (c + 1) * 128, :])
                    wts[(nm, c)] = t
            b1t = cpool.tile([128, 2], F32, name="b1t")
            nc.sync.dma_start(b1t[:, :], b1[:, :])
            b2t = cpool.tile([128, 2], F32, name="b2t")
            nc.sync.dma_start(b2t[:, :], b2[:, :])
            bep1t = cpool.tile([128, 2], F32, name="bep1t")
            nc.sync.dma_start(bep1t[:, :], bep1[:, :])
            wep2t = cpool.tile([128, 2], E_DT, name="wep2t")
            if E_DT == F32:
                nc.sync.dma_start(wep2t[:, :], wep2[:, :])
            else:
                nc.gpsimd.dma_start(wep2t[:, :], wep2[:, :])  # casts
            bep2t = cpool.tile([128, 1], F32, name="bep2t")
            nc.sync.dma_start(bep2t[:, :], bep2[:, :])

            ppp = tc.alloc_tile_pool(name="psum_pair", bufs=1, space="PSUM")
            # 8 pairwise psum banks; generation g covers rows 32g..32g+31;
            # bank kk gets rows 32g+4kk+u at partition 32u.
            banks = [ppp.tile([128, N], F32, name=f"bank{kk}") for kk in range(8)]
            # zero banks once so the sigmoid's garbage partitions are finite
            for kk in range(8):
                nc.vector.memset(banks[kk][:, :], 0.0)

            for _rep in range(reps):
                _body(nc, wpool, epool, scpool, banks,
                      xt, y, wts, b1t, b2t, bep1t, wep2t, bep2t, NT)

            ppp.release()

    return nc


_NC_CACHE = {}


def _get_nc(reps: int = 1):
    if reps not in _NC_CACHE:
        _NC_CACHE[reps] = build_nc(reps)
    return _NC_CACHE[reps]


def make_in_maps(step_sequence, step_mask, W_gc1, b_gc1, W_gc2, b_gc2,
                 W_ep1, b_ep1, w_ep2, b_ep2):
    x = np.ascontiguousarray(np.asarray(step_sequence, dtype=np.float32))
    W_gc1 = np.asarray(W_gc1, np.float32)
    W_gc2 = np.asarray(W_gc2, np.float32)
    W_ep1 = np.asarray(W_ep1, np.float32)
    b_gc1 = np.asarray(b_gc1, np.float32)
    b_gc2 = np.asarray(b_gc2, np.float32)
    b_ep1 = np.asarray(b_ep1, np.float32)
    w_ep2 = np.asarray(w_ep2, np.float32)
    b_ep2v = np.float32(np.asarray(b_ep2))

    w1t = np.ascontiguousarray(W_gc1.T)
    w2t = np.ascontiguousarray(W_gc2.T)
    wat = np.ascontiguousarray(W_ep1[:, :H].T)
    wbt = np.ascontiguousarray(W_ep1[:, H:].T)
    b1m = np.ascontiguousarray(b_gc1.reshape(2, 128).T)
    b2m = np.ascontiguousarray(b_gc2.reshape(2, 128).T)
    bep1m = np.ascontiguousarray(b_ep1.reshape(2, 128).T)
    wep2m = np.ascontiguousarray(w_ep2.reshape(2, 128).T)
    bep2m = np.full((128, 1), b_ep2v, np.float32)

    in_maps = []
    for d in range(NCORES):
        b, ph = divmod(d, 4)
        my_i = np.arange(ph, N, 4)
        xT = x[b].T  # [H, N]
        xTmy = np.ascontiguousarray(x[b][my_i].T)  # [H, R]
        xt640 = np.ascontiguousarray(np.concatenate([xT, xTmy], axis=1))
        in_maps.append({
            "xt": xt640, "w1t": w1t, "w2t": w2t, "wat": wat, "wbt": wbt,
            "b1": b1m, "b2": b2m, "bep1": bep1m, "wep2": wep2m, "bep2": bep2m,
        })
    return in_maps


_MASK_CACHE = {}


def _tril_mask():
    if "m" not in _MASK_CACHE:
        _MASK_CACHE["m"] = np.tril(np.ones((N, N), np.float32), k=-1)
    return _MASK_CACHE["m"]


def gather_output(results):
    out = np.zeros((B, N, N), np.float32)
    for d in range(NCORES):
        b, ph = divmod(d, 4)
        dev = results[d]["y"]  # [R, N]
        for lvl in range(8):
            J = 64 * (lvl + 1)
            ks = np.arange(16 * lvl, 16 * (lvl + 1))
            out[b, 4 * ks + ph, :J] = dev[16 * lvl : 16 * (lvl + 1), :J]
    out *= _tril_mask()[None, :, :]
    return out


def kernel(**inputs) -> np.ndarray:
    nc = _get_nc()
    in_maps = make_in_maps(**inputs)
    res = run_bass_kernel_spmd(nc, in_maps, core_ids=list(range(NCORES)))
    return gather_output(res.results)


# revision 15
# speedup vs baseline: 11.8164x; 11.8164x over previous
"""Trainium2 Bass kernel for CausalGraphNetwork.

Computes, for x = step_sequence [B=2, N=512, H=256]:
    h  = relu(x @ W_gc1.T + b_gc1)
    f  = relu(h @ W_gc2.T + b_gc2)
    a  = f @ Wa.T            (Wa = W_ep1[:, :H])
    c  = f @ Wb.T            (Wb = W_ep1[:, H:])
    e[b,i,j,:] = relu(a[b,i,:] + c[b,j,:] + b_ep1)
    scores = sigmoid(e @ w_ep2 + b_ep2) * strict_lower_mask

Strategy (8 NeuronCores, SPMD single program):
  - Core d owns batch d//4, rows i = 4k + (d%4), k = 0..127.  The
    interleaved row assignment makes the causal work profile identical
    on every core, so one program serves all cores.
  - Everything on-chip in transposed layout (h on partitions):
    hT/fT/cT computed with fp32 matmuls; per row, e-chunks are built
    with a per-partition-bias tensor_scalar (relu fused via op1=max)
    and reduced over h by M=1 matmuls (lhsT = w_ep2 chunk) into PSUM.
  - PSUM writes for M=1 must land on 32-aligned partitions, so 4 rows
    share a bank at partitions {0,32,64,96}; sigmoid (ACT) drains a
    whole bank to SBUF, and a strided-partition DMA extracts the rows.
  - Causality: row k only computes j < Jext(k) = 64*ceil((k+1)/16);
    the host applies the exact strict-lower mask after gathering.
"""

import math

import ml_dtypes
import numpy as np

import bass_rust
import concourse.bass as bass
import concourse.mybir as mybir
import concourse.tile as tile
from concourse.bass_utils import run_bass_kernel_spmd
from concourse.vector_clock import ScopedClock

B, N, H = 2, 512, 256
NCORES = 8
R = 128  # rows per core
F32 = mybir.dt.float32
BF16 = mybir.dt.bfloat16
AF = mybir.ActivationFunctionType
ALU = mybir.AluOpType

# e-tile dtype for the pairwise stage
E_DT = BF16
# engine for e-generation per h-chunk: "vector", "gpsimd", or "scalar"
E_ENGINES = ("vector", "vector")
# chunk-1 e-gens whose k%8 is in ACT_ROWS run on ACT instead of DVE
ACT_ROWS = ()
# which engine queue carries the xt input DMA
XT_ON_ACT = False


def jext(k: int) -> int:
    """Causal column extent for local row k (covers i = 4k+3 worst case)."""
    return min(N, 64 * math.ceil((k + 1) / 16))


def jbx(k: int) -> int:
    """Exact-ish per-row compute extent (multiple of 8, >= 4k+4)."""
    return min(N, ((4 * k + 4) + 7) // 8 * 8)


class _TC(tile.TileContext):
    """TileContext variant for a walrus build that only supports ONE sem
    wait per instruction: split multi-wait instructions by hoisting the
    extra waits onto NOPs inserted just before them."""

    MAXW = 1

    def _split_waits_in_list(self, insts):
        out = []
        for inst in insts:
            si = inst.sync_info
            waits = list(si.on_wait) if si is not None else []
            if len(waits) > self.MAXW:
                rest, keep = waits[: -self.MAXW], waits[-self.MAXW :]
                for i in range(0, len(rest), self.MAXW):
                    nop = mybir.InstNoOp(
                        name=self.nc.get_next_instruction_name(),
                        engine=inst.engine,
                        bass_nofuse=True,
                        sync_info=bass_rust.SyncInfo(
                            on_wait=rest[i : i + self.MAXW], on_update=[]
                        ),
                    )
                    out.append(nop)
                inst.sync_info = bass_rust.SyncInfo(
                    on_wait=keep, on_update=list(si.on_update)
                )
            out.append(inst)
        return out

    def _lower_ordered_insts(self, ordered):
        for bb_name in list(ordered.keys()):
            ordered[bb_name] = self._split_waits_in_list(ordered[bb_name])
        return super()._lower_ordered_insts(ordered)

    def _drain_and_barrier(self, tick_clock, wait_clock):
        drain_inst = self.nc.sync.drain()
        wait_clock.add_sem_waits(
            drain_inst.ins, ScopedClock({None: tick_clock.global_clock})
        )
        si = drain_inst.ins.sync_info
        waits = list(si.on_wait) if si is not None else []
        if len(waits) > self.MAXW:
            drain_inst.ins.sync_info = bass_rust.SyncInfo(
                on_wait=waits[: self.MAXW], on_update=list(si.on_update)
            )
            rest = waits[self.MAXW :]
            for i in range(0, len(rest), self.MAXW):
                nop = self.nc.sync.nop(nofuse=True, hint=f"dw{i}")
                nop.ins.sync_info = bass_rust.SyncInfo(
                    on_wait=rest[i : i + self.MAXW], on_update=[]
                )
        self.nc.all_engine_barrier()
        assert self.sems is not None
        popped = self.nc._tile_sem_poison_stack.pop()
        assert popped is self._sem_poison
        self.nc.clear_and_free_semaphores(list(self.sems.allocated().values()))
        self.nc.all_engine_barrier()


def _egen(nc, engine: str, out_ap, in_ap, bias_col):
    """e = relu(in + bias) with per-partition bias, on the chosen engine."""
    if engine == "vector":
        nc.vector.tensor_scalar(
            out=out_ap, in0=in_ap, scalar1=bias_col, scalar2=0.0,
            op0=ALU.add, op1=ALU.max,
        )
    elif engine == "gpsimd":
        nc.gpsimd.tensor_scalar(
            out=out_ap, in0=in_ap, scalar1=bias_col, scalar2=0.0,
            op0=ALU.add, op1=ALU.max,
        )
    elif engine == "scalar":
        nc.scalar.activation(out_ap, in_ap, AF.Relu, bias=bias_col)
    else:
        raise ValueError(engine)


def _body(nc, wpool, epool, scpool, banks,
          xt, y, wts, b1t, b2t, bep1t, wep2t, bep2t, NT,
          engines=None, skip_pairwise=False, skip_upstream=False):
    if engines is None:
        engines = E_ENGINES
    bank_rr = [0]

    def next_bank(tn):
        kk = bank_rr[0] % 8
        bank_rr[0] += 1
        return banks[kk][:, 0:tn]  # banks[kk] is an AP view into a quad
    xts = []
    for c in range(2):
        t = wpool.tile([128, NT], BF16, name=f"xt_{c}", tag=f"xt_{c}")
        # issue on the ACT queue to keep the SP queue free for output DMAs
        dma_eng = nc.scalar if XT_ON_ACT else nc.sync
        dma_eng.dma_start(t[:, :], xt[c * 128 : (c + 1) * 128, :])
        xts.append(t)

    # ---- upstream: hT -> fT -> (cT, aTp), all [o-chunk, tokens] ----
    def layer(dst_tiles, src_tiles, wname, bias_tile, func, tslices):
        for oc in range(2):
            for (t0, tn) in tslices:
                ps = next_bank(tn)
                for kc in range(2):
                    nc.tensor.matmul(
                        ps,
                        lhsT=wts[(wname, kc)][:, oc * 128 : (oc + 1) * 128],
                        rhs=src_tiles[kc][:, t0 : t0 + tn],
                        start=(kc == 0), stop=(kc == 1),
                    )
                dst = dst_tiles[oc][:, t0 : t0 + tn]
                bias_col = bias_tile[:, oc : oc + 1] if bias_tile is not None else None
                if oc == 0:
                    if func == "relu":
                        nc.scalar.activation(dst, ps, AF.Relu, bias=bias_col)
                    elif func == "addbias":
                        nc.scalar.activation(dst, ps, AF.Identity, bias=bias_col)
                    else:
                        nc.scalar.copy(dst, ps)
                else:
                    if func == "relu":
                        nc.scalar.activation(dst, ps, AF.Relu, bias=bias_col)
                    elif func == "addbias":
                        nc.scalar.activation(dst, ps, AF.Identity, bias=bias_col)
                    else:
                        nc.scalar.copy(dst, ps)

    hts = [wpool.tile([128, NT], BF16, name=f"ht_{c}", tag=f"ht_{c}") for c in range(2)]
    if skip_upstream:
        cts = [wpool.tile([128, N], E_DT, name=f"ct_{c}", tag=f"ct_{c}") for c in range(2)]
        ats = [wpool.tile([128, R], F32, name=f"at_{c}", tag=f"at_{c}") for c in range(2)]
        for c in range(2):
            nc.vector.memset(cts[c][:, :], 0.1)
            nc.vector.memset(ats[c][:, :], 0.1)
        _pairwise(nc, epool, scpool, banks, y, cts, ats, wep2t, bep2t, engines,
                  skip_pairwise)
        return
    layer(hts, xts, "w1t", b1t, "relu", [(0, 512), (512, R)])
    fts = [wpool.tile([128, NT], BF16, name=f"ft_{c}", tag=f"ft_{c}") for c in range(2)]
    layer(fts, hts, "w2t", b2t, "relu", [(0, 512), (512, R)])
    cts = [wpool.tile([128, N], E_DT, name=f"ct_{c}", tag=f"ct_{c}") for c in range(2)]
    layer(cts, fts, "wbt", None, "copy", [(0, 512)])

    # aTp = Wa-projection of own tokens + b_ep1
    ats = [wpool.tile([128, R], F32, name=f"at_{c}", tag=f"at_{c}") for c in range(2)]
    for oc in range(2):
        ps = next_bank(R)
        for kc in range(2):
            nc.tensor.matmul(
                ps,
                lhsT=wts[("wat", kc)][:, oc * 128 : (oc + 1) * 128],
                rhs=fts[kc][:, 512 : 512 + R],
                start=(kc == 0), stop=(kc == 1),
            )
        dst = ats[oc][:, :]
        bias_col = bep1t[:, oc : oc + 1]
        nc.scalar.activation(dst, ps, AF.Identity, bias=bias_col)

    _pairwise(nc, epool, scpool, banks, y, cts, ats, wep2t, bep2t, engines,
              skip_pairwise)


def _pairwise(nc, epool, scpool, banks, y, cts, ats, wep2t, bep2t, engines,
              skip_pairwise):
    if skip_pairwise:
        return
    # ---- pairwise stage ----
    # banks[kk] is a [128, 512] column-slice view of a quad psum tile; quad
    # q covers banks 4q..4q+3 and rows 32g+16q..+15 in generation g.
    for g in range(4):
        for q in range(2):
            for kk in range(4 * q, 4 * q + 4):
                ps = banks[kk]
                for u in range(4):
                    k = 32 * g + 4 * kk + u
                    jb = jbx(k)
                    for c in range(2):
                        e = epool.tile([128, N], E_DT, name=f"e{c}",
                                       tag=f"e{c}", bufs=4)
                        eng = engines[c]
                        if c == 1 and (k % 8) in ACT_ROWS:
                            eng = "scalar"
                        _egen(nc, eng,
                              e[:, 0:jb], cts[c][:, 0:jb], ats[c][:, k : k + 1])
                        nc.tensor.matmul(
                            ps[32 * u : 32 * u + 1, 0:jb],
                            lhsT=wep2t[:, c : c + 1],
                            rhs=e[:, 0:jb],
                            start=(c == 0), stop=(c == 1),
                            tile_position=(0, 32 * u),
                        )
            # quad epilogue: one sigmoid + one DMA for 4 banks (16 rows)
            jbq = min(N, 64 * (2 * g + q + 1))
            quad = banks[4 * q].tensor  # the quad tile backing these slices
            qin = quad.ap().rearrange("p (kk j) -> p kk j", kk=4)[:, :, 0:jbq]
            sc = scpool.tile([128, 4 * N], F32, name="sc", tag="sc")
            qout = sc.rearrange("p (kk j) -> p kk j", kk=4)[:, :, 0:jbq]
            nc.scalar.activation(qout, qin, AF.Sigmoid, bias=bep2t[:, 0:1])
            r0 = 32 * g + 16 * q
            dst = y[r0 : r0 + 16, 0:jbq].rearrange("(kk u) j -> u kk j", u=4)
            src = qout[0:128:32, :, :]
            nc.sync.dma_start(dst, src)


def build_nc(reps: int = 1, engines=None, skip_pairwise=False,
             skip_upstream=False) -> bass.Bass:
    nc = bass.Bass("TRN2", target_bir_lowering=False, debug=False)

    NT = N + R  # 640 token columns: 512 shared j-tokens + 128 own i-tokens

    xt = nc.dram_tensor("xt", [H, NT], BF16, kind="ExternalInput")
    w1t = nc.dram_tensor("w1t", [H, H], BF16, kind="ExternalInput")
    w2t = nc.dram_tensor("w2t", [H, H], BF16, kind="ExternalInput")
    wat = nc.dram_tensor("wat", [H, H], BF16, kind="ExternalInput")
    wbt = nc.dram_tensor("wbt", [H, H], BF16, kind="ExternalInput")
    b1 = nc.dram_tensor("b1", [128, 2], F32, kind="ExternalInput")
    b2 = nc.dram_tensor("b2", [128, 2], F32, kind="ExternalInput")
    bep1 = nc.dram_tensor("bep1", [128, 2], F32, kind="ExternalInput")
    wep2 = nc.dram_tensor("wep2", [128, 2], E_DT, kind="ExternalInput")
    bep2 = nc.dram_tensor("bep2", [128, 1], F32, kind="ExternalInput")
    y = nc.dram_tensor("y", [R, N], F32, kind="ExternalOutput")

    with _TC(nc) as tc:
        with tc.tile_pool(name="const", bufs=1) as cpool, \
             tc.tile_pool(name="work", bufs=2) as wpool, \
             tc.tile_pool(name="epool", bufs=3) as epool, \
             tc.tile_pool(name="scpool", bufs=4) as scpool:

            # ---- constants (loaded once, reused across reps) ----
            # spread the weight loads across engine queues so the startup
            # DMAs run in parallel
            wts = {}
            qengs = [nc.sync, nc.scalar, nc.sync, nc.scalar]
            for qi, (nm, dram) in enumerate(
                (("w1t", w1t), ("w2t", w2t), ("wat", wat), ("wbt", wbt))
            ):
                for c in range(2):
                    t = cpool.tile([128, H], BF16, name=f"{nm}_{c}")
                    qengs[qi].dma_start(t[:, :], dram[c * 128 : (c + 1) * 128, :])
                    wts[(nm, c)] = t
            b1t = cpool.tile([128, 2], F32, name="b1t")
            nc.sync.dma_start(b1t[:, :], b1[:, :])
            b2t = cpool.tile([128, 2], F32, name="b2t")
            nc.sync.dma_start(b2t[:, :], b2[:, :])
            bep1t = cpool.tile([128, 2], F32, name="bep1t")
            nc.sync.dma_start(bep1t[:, :], bep1[:, :])
            wep2t = cpool.tile([128, 2], E_DT, name="wep2t")
            nc.sync.dma_start(wep2t[:, :], wep2[:, :])
            bep2t = cpool.tile([128, 1], F32, name="bep2t")
            nc.sync.dma_start(bep2t[:, :], bep2[:, :])

            ppp = tc.alloc_tile_pool(name="psum_pair", bufs=1, space="PSUM")
            # two quad psum tiles (4 banks each); banks[kk] is a 512-col view
            quads = [ppp.tile([128, 4 * N], F32, name=f"quad{q}")
                     for q in range(2)]
            banks = [quads[kk // 4][:, 512 * (kk % 4) : 512 * (kk % 4) + 512]
                     for kk in range(8)]
            # zero banks once (PE start=True against a zero operand) so the
            # sigmoid's garbage partitions are finite
            zlhs = cpool.tile([128, 128], BF16, name="zlhs")
            zrhs = cpool.tile([128, N], BF16, name="zrhs")
            nc.vector.memset(zlhs[:, :], 0.0)
            nc.vector.memset(zrhs[:, :], 0.0)
            for kk in range(8):
                nc.tensor.matmul(banks[kk], lhsT=zlhs[:, :],
                                 rhs=zrhs[:, :], start=True, stop=True)

            for _rep in range(reps):
                _body(nc, wpool, epool, scpool, banks,
                      xt, y, wts, b1t, b2t, bep1t, wep2t, bep2t, NT,
                      engines=engines, skip_pairwise=skip_pairwise,
                      skip_upstream=skip_upstream)

            ppp.release()

    return nc


_NC_CACHE = {}


def _get_nc(reps: int = 1):
    if reps not in _NC_CACHE:
        _NC_CACHE[reps] = build_nc(reps)
    return _NC_CACHE[reps]


def make_in_maps(step_sequence, step_mask, W_gc1, b_gc1, W_gc2, b_gc2,
                 W_ep1, b_ep1, w_ep2, b_ep2):
    x = np.ascontiguousarray(np.asarray(step_sequence, dtype=np.float32))
    W_gc1 = np.asarray(W_gc1, np.float32)
    W_gc2 = np.asarray(W_gc2, np.float32)
    W_ep1 = np.asarray(W_ep1, np.float32)
    b_gc1 = np.asarray(b_gc1, np.float32)
    b_gc2 = np.asarray(b_gc2, np.float32)
    b_ep1 = np.asarray(b_ep1, np.float32)
    w_ep2 = np.asarray(w_ep2, np.float32)
    b_ep2v = np.float32(np.asarray(b_ep2))

    bf16 = ml_dtypes.bfloat16
    w1t = np.ascontiguousarray(W_gc1.T).astype(bf16)
    w2t = np.ascontiguousarray(W_gc2.T).astype(bf16)
    wat = np.ascontiguousarray(W_ep1[:, :H].T).astype(bf16)
    wbt = np.ascontiguousarray(W_ep1[:, H:].T).astype(bf16)
    b1m = np.ascontiguousarray(b_gc1.reshape(2, 128).T)
    b2m = np.ascontiguousarray(b_gc2.reshape(2, 128).T)
    bep1m = np.ascontiguousarray(b_ep1.reshape(2, 128).T)
    wep2m = np.ascontiguousarray(w_ep2.reshape(2, 128).T)
    if E_DT == BF16:
        wep2m = wep2m.astype(bf16)
    bep2m = np.full((128, 1), b_ep2v, np.float32)

    in_maps = []
    for d in range(NCORES):
        b, ph = divmod(d, 4)
        my_i = np.arange(ph, N, 4)
        xT = x[b].T  # [H, N]
        xTmy = np.ascontiguousarray(x[b][my_i].T)  # [H, R]
        xt640 = np.ascontiguousarray(
            np.concatenate([xT, xTmy], axis=1)).astype(bf16)
        in_maps.append({
            "xt": xt640, "w1t": w1t, "w2t": w2t, "wat": wat, "wbt": wbt,
            "b1": b1m, "b2": b2m, "bep1": bep1m, "wep2": wep2m, "bep2": bep2m,
        })
    return in_maps


_MASK_CACHE = {}


def _tril_mask():
    if "m" not in _MASK_CACHE:
        _MASK_CACHE["m"] = np.tril(np.ones((N, N), np.float32), k=-1)
    return _MASK_CACHE["m"]


def gather_output(results):
    out = np.zeros((B, N, N), np.float32)
    for d in range(NCORES):
        b, ph = divmod(d, 4)
        dev = results[d]["y"]  # [R, N]
        for lvl in range(8):
            J = 64 * (lvl + 1)
            ks = np.arange(16 * lvl, 16 * (lvl + 1))
            out[b, 4 * ks + ph, :J] = dev[16 * lvl : 16 * (lvl + 1), :J]
    out *= _tril_mask()[None, :, :]
    return out


def kernel(**inputs) -> np.ndarray:
    nc = _get_nc()
    in_maps = make_in_maps(**inputs)
    res = run_bass_kernel_spmd(nc, in_maps, core_ids=list(range(NCORES)))
    return gather_output(res.results)
